# revision 13
# baseline (speedup 1.0000x reference)
"""CBOW (one-hot embedding lookup + mean + output matmul + softmax) on 8
Trainium2 NeuronCores, vocab-sharded end to end. v2.

Full problem: batch [1024, 10, 32000] f32 one-hot, emb [32000, 128] f32,
w_out [128, 32000] f32 -> softmax(mean_c(batch @ emb) @ w_out) [1024, 32000].

Sharding: core i owns vocab columns [i*4000, (i+1)*4000). It receives
  batch_s [1024, 10, 4000] f32  (full batch, its vocab slice)
  emb_s   [4096, 128]      f32  (its emb rows, zero-padded 4000->4096)
  w_out_s [128, 4000]      f32  (its output-projection columns)
and produces out_s [1024, 4000] bf16 (its softmax columns; host concatenates
along vocab and upcasts to f32).

v2 design vs v1:
- Stage-1 streams full-vocab-width context slabs [128, 2, 4000] f32 on the
  sync HWDGE ring: 32 KB contiguous per partition line (vs 4 KB segments),
  one descriptor per partition, far lower issue cost per byte.
- The 10-plane context sum moves off the PE onto DVE pair-adds in bf16
  (one-hot sums <= 10 are bf16-exact): per block 5 pair-adds + 4
  accumulates instead of 320 transpose matmuls. PE per block: 32 bf16
  transpose matmuls + 32 fp32 emb matmuls.
- Block 7's last slab is split into shrinking vocab-region DMAs so the
  final region's chain (pair-add -> acc -> PE tiles -> park) starts before
  the whole slab lands, shrinking the post-stream critical path.
- Collectives (gpsimd-triggered, serial CC stream): 5 instead of 8.
  Wave w's softmax denominator rides wave w+1's avg AllReduce, so the
  post-stream tail is just: avg+den AR (idle ~13us) -> block-7 logits/exp
  -> den AR -> scale+store.
- Scalar engine runs only Exp/Reciprocal activations + DMA issue (CC
  bounces, readbacks, outputs); softmax scaling runs on gpsimd during the
  stream (so the in-order DVE queue never waits on a collective) and on
  DVE in the tail (where DVE is idle and gpsimd is busy triggering).
"""

from contextlib import ExitStack

import numpy as np

import concourse.bass as bass
import concourse.tile as tile
from concourse import bacc, masks, mybir
from concourse._compat import with_exitstack

F32 = mybir.dt.float32
BF16 = mybir.dt.bfloat16
AX = mybir.AxisListType
AF = mybir.ActivationFunctionType
ALU = mybir.AluOpType

B_FULL, C, V, D = 1024, 10, 32000, 128
N_CORES = 8
VS = V // N_CORES          # 4000 vocab columns per core
VS_PAD = 4096              # emb rows padded to a multiple of 128
N_TILES = VS_PAD // 128    # 32 v-tiles (last is 32 valid rows)
BB = 128                   # batch rows per block
N_BB = B_FULL // BB        # 8 blocks
NC2 = 512                  # stage-2 logits chunk (fp32 moving-operand max)
# slab-4 vocab-region split: the last slab streams region-by-region so the
# block-end chain (pair-add -> acc -> PE tiles -> park) pipelines with the
# DMA instead of serializing after it.  Block 7 gets an extra-fine tail.
MID_REGIONS = [(0, 1280), (1280, 2560), (2560, 3456), (3456, VS)]
TAIL_REGIONS = [(0, 1280), (1280, 2560), (2560, 3456), (3456, 3840), (3840, VS)]

# wave -> (first block, last block+1); den of waves 0/1 rides the next
# wave's avg AR; dens of waves 2/3 go in one small post-stream AR.
WAVES = [(0, 3), (3, 5), (5, 7), (7, 8)]
N_W = len(WAVES)


@with_exitstack
def _cbow_kernel(ctx: ExitStack, tc, out, batch, emb, w_out):
    nc = tc.nc
    Bs, Cs, Vs = batch.shape
    assert Bs == B_FULL and Cs == C and Vs == VS
    rg = [list(range(N_CORES))]
    n_nc = (Vs + NC2 - 1) // NC2

    const_pool = ctx.enter_context(tc.tile_pool(name="const", bufs=1))
    ident = const_pool.tile([128, 128], BF16)
    masks.make_identity(nc, ident[:])

    def warmup_cc():
        """Tiny throwaway AllReduce: absorbs the ~11us first-trigger CC
        startup so the first real collective starts promptly."""
        wm_in = dram.tile([128, 1], F32, tag="wm_in", bufs=1)
        wm_out = dram.tile(
            [128, 1], F32, tag="wm_out", addr_space="Shared", bufs=1
        )
        wm_sb = stat_pool.tile([128, 1], F32, tag="wm_sb")
        nc.vector.memset(wm_sb[:], 0.0)
        nc.scalar.dma_start(wm_in[:], wm_sb[:])
        nc.gpsimd.collective_compute(
            "AllReduce",
            ALU.add,
            replica_groups=rg,
            ins=[wm_in.opt()],
            outs=[wm_out.opt()],
        )

    eb_pool = ctx.enter_context(tc.tile_pool(name="eb", bufs=1))
    eb = eb_pool.tile([128, N_TILES, 128], F32)
    wo_pool = ctx.enter_context(tc.tile_pool(name="wo", bufs=1))
    wo = wo_pool.tile([128, VS], F32)

    slab_pool = ctx.enter_context(tc.tile_pool(name="slab", bufs=2))
    tmp_pool = ctx.enter_context(tc.tile_pool(name="tmp", bufs=2))
    # 3 acc bufs: a PE stall (e.g. stage2a matmuls hoisted ahead of a slow
    # collective readback) must not block DVE's stream adds via recycling
    acc_pool = ctx.enter_context(tc.tile_pool(name="acc", bufs=3))
    sT_pool = ctx.enter_context(tc.tile_pool(name="sT", bufs=3))
    sTps_pool = ctx.enter_context(tc.tile_pool(name="sTps", bufs=3, space="PSUM"))
    avgps_pool = ctx.enter_context(tc.tile_pool(name="avgps", bufs=2, space="PSUM"))
    avgsb_pool = ctx.enter_context(tc.tile_pool(name="avgsb", bufs=1))
    rb_pool = ctx.enter_context(tc.tile_pool(name="rb", bufs=1))
    lg_pool = ctx.enter_context(tc.tile_pool(name="lg", bufs=5))
    lgps_pool = ctx.enter_context(tc.tile_pool(name="lgps", bufs=3, space="PSUM"))
    stat_pool = ctx.enter_context(tc.tile_pool(name="stat", bufs=2))
    dram = ctx.enter_context(tc.tile_pool(name="dram", bufs=2, space="DRAM"))

    bb2wave = {}
    wstate = {}
    cur = {"acc": None}
    for wi, (w0, w1) in enumerate(WAVES):
        nb = w1 - w0
        wstate[wi] = {
            "w0": w0,
            "w1": w1,
            "nb": nb,
            "avg_sb": avgsb_pool.tile(
                [128, nb * BB], F32, tag=f"avgsb{wi}", name=f"avgsb{wi}"
            ),
            "den_sb": stat_pool.tile(
                [128, nb], F32, tag=f"densb{wi}", name=f"densb{wi}"
            ),
            "lg": {},
            "sums": {},
        }
        for bb in range(w0, w1):
            bb2wave[bb] = wi

    def pe_tiles(acc, avgT_ps, r0, r1):
        """Transpose + emb matmuls for v-tiles covering [r0, r1)."""
        for toff in range(r0, r1, 128):
            tw = min(128, Vs - toff)
            g = toff // 128
            sT_ps = sTps_pool.tile([128, BB], F32, tag="sTps")
            nc.tensor.matmul(
                sT_ps[:tw],
                lhsT=acc[:, toff : toff + tw],
                rhs=ident[:],
                start=True,
                stop=True,
            )
            sT = sT_pool.tile([128, BB], F32, tag="sT")
            # PSUM->SBUF bounce on scalar: keeps DVE free for the stream adds
            nc.scalar.copy(sT[:tw], sT_ps[:tw])
            nc.tensor.matmul(
                avgT_ps[:],
                lhsT=eb[:tw, g, :],
                rhs=sT[:tw],
                start=(g == 0),
                stop=(g == N_TILES - 1),
            )

    def park(bb, avgT_ps):
        wi = bb2wave[bb]
        st = wstate[wi]
        slot = bb - st["w0"]
        nc.scalar.copy(
            st["avg_sb"][:, slot * BB : (slot + 1) * BB], avgT_ps[:]
        )

    def stage1_slab(bb, s):
        """Stream slab s (context planes 2s, 2s+1) of block bb and fold it
        into the block's bf16 context-sum; slab 4 also runs the PE tiles
        and parks the block's avgT."""
        b0 = bb * BB
        if s == 0:
            slab = slab_pool.tile([128, 2, Vs], F32, tag="slab")
            nc.sync.dma_start(slab[:], batch[b0 : b0 + BB, 0:2, :])
            acc = acc_pool.tile([128, Vs], BF16, tag="acc")
            nc.vector.tensor_tensor(acc[:], slab[:, 0, :], slab[:, 1, :], ALU.add)
            cur["acc"] = acc
            return
        acc = cur["acc"]
        if s < 4:
            slab = slab_pool.tile([128, 2, Vs], F32, tag="slab")
            nc.sync.dma_start(slab[:], batch[b0 : b0 + BB, 2 * s : 2 * s + 2, :])
            tmpv = tmp_pool.tile([128, Vs], BF16, tag="tmp")
            nc.vector.tensor_tensor(tmpv[:], slab[:, 0, :], slab[:, 1, :], ALU.add)
            nc.vector.tensor_add(acc[:], acc[:], tmpv[:])
            return
        # s == 4: last slab streams region-by-region; PE tiles + park chase it
        slab = slab_pool.tile([128, 2, Vs], F32, tag="slab")
        tmpv = tmp_pool.tile([128, Vs], BF16, tag="tmp")
        avgT_ps = avgps_pool.tile([128, BB], F32, tag="avgT")
        regions = TAIL_REGIONS if bb == N_BB - 1 else MID_REGIONS
        for r0, r1 in regions:
            nc.sync.dma_start(
                slab[:, :, r0:r1], batch[b0 : b0 + BB, 8:10, r0:r1]
            )
            nc.vector.tensor_tensor(
                tmpv[:, r0:r1], slab[:, 0, r0:r1], slab[:, 1, r0:r1], ALU.add
            )
            nc.vector.tensor_add(acc[:, r0:r1], acc[:, r0:r1], tmpv[:, r0:r1])
            pe_tiles(acc, avgT_ps, r0, r1)
        park(bb, avgT_ps)

    def run_ar(wi, ride_den=None, bounce_eng=None):
        """AllReduce wave wi's avg partials; optionally an earlier wave's
        denominators ride along.  Bounce via DRAM (scalar mid-stream, sync
        in the tail); trigger on gpsimd; readback also on gpsimd — the
        trigger already blocks gpsimd until the collective completes, so
        the readback issues the moment the data is ready."""
        be = bounce_eng or nc.scalar
        st = wstate[wi]
        nb = st["nb"]
        cols = nb * BB
        extra = wstate[ride_den]["nb"] if ride_den is not None else 0
        cc_in = dram.tile([128, cols + extra], F32, tag=f"ccin{wi}", bufs=1)
        cc_out = dram.tile(
            [128, cols + extra],
            F32,
            tag=f"ccout{wi}",
            addr_space="Shared",
            bufs=1,
        )
        be.dma_start(cc_in[:, :cols], st["avg_sb"][:])
        if extra:
            be.dma_start(cc_in[:, cols:], wstate[ride_den]["den_sb"][:])
        nc.gpsimd.collective_compute(
            "AllReduce",
            ALU.add,
            replica_groups=rg,
            ins=[cc_in.opt()],
            outs=[cc_out.opt()],
        )
        rb = rb_pool.tile([128, cols + extra], F32, tag=f"rb{wi}")
        nc.gpsimd.dma_start(rb[:], cc_out[:])
        st["avg_g"] = rb
        if extra:
            wstate[ride_den]["den_g"] = rb[:, cols:]

    def run_final_den_ar():
        """One small post-stream AllReduce for waves 2+3 denominators."""
        nbt = wstate[2]["nb"] + wstate[3]["nb"]
        cc_in = dram.tile([128, nbt], F32, tag="ccin_fin", bufs=1)
        cc_out = dram.tile(
            [128, nbt], F32, tag="ccout_fin", addr_space="Shared", bufs=1
        )
        nc.sync.dma_start(cc_in[:, : wstate[2]["nb"]], wstate[2]["den_sb"][:])
        nc.sync.dma_start(cc_in[:, wstate[2]["nb"] :], wstate[3]["den_sb"][:])
        nc.gpsimd.collective_compute(
            "AllReduce",
            ALU.add,
            replica_groups=rg,
            ins=[cc_in.opt()],
            outs=[cc_out.opt()],
        )
        rb = rb_pool.tile([128, nbt], F32, tag="rb_fin")
        nc.gpsimd.dma_start(rb[:], cc_out[:])
        wstate[2]["den_g"] = rb[:, : wstate[2]["nb"]]
        wstate[3]["den_g"] = rb[:, wstate[2]["nb"] :]

    def stage2a(bb, collapse=True):
        """Logits + exp (fused row-sum) + this block's local denominator."""
        wi = bb2wave[bb]
        st = wstate[wi]
        slot = bb - st["w0"]
        avg_g = st["avg_g"]
        lg = lg_pool.tile([128, VS], BF16, tag="lg")
        sums = stat_pool.tile([128, n_nc], F32, tag="sums")
        for k in range(n_nc):
            n0 = k * NC2
            nw = min(NC2, Vs - n0)
            lg_ps = lgps_pool.tile([128, NC2], F32, tag="lgps")
            nc.tensor.matmul(
                lg_ps[:, :nw],
                lhsT=avg_g[:, slot * BB : (slot + 1) * BB],
                rhs=wo[:, n0 : n0 + nw],
                start=True,
                stop=True,
            )
            # exp(x / C): folds the 1/C mean; fused row-sum via accum_out
            nc.scalar.activation(
                lg[:, n0 : n0 + nw],
                lg_ps[:, :nw],
                AF.Exp,
                scale=1.0 / Cs,
                accum_out=sums[:, k : k + 1],
            )
        st["lg"][bb] = lg
        st["sums"][bb] = sums
        if collapse:
            den_collapse(bb)

    def den_collapse(bb):
        """Sum the 8 chunk row-sums into this block's local denominator
        (DVE; deferred for blocks whose exps finish near stream end)."""
        wi = bb2wave[bb]
        st = wstate[wi]
        slot = bb - st["w0"]
        sums = st["sums"].pop(bb)
        nc.vector.tensor_reduce(
            st["den_sb"][:, slot : slot + 1], sums[:], AX.X, ALU.add
        )

    def stage2b(wi, scales_on_scalar=True):
        """Global denominator -> reciprocal (DVE, emitted only at slots
        where the collective result has surely landed) -> scale -> out.
        Mid-stream waves scale on the scalar engine's ACT mul so the DVE
        stream chain stays short; tail waves scale on the then-idle DVE."""
        st = wstate[wi]
        nb = st["nb"]
        r = stat_pool.tile([128, nb], F32, tag=f"recip{wi}")
        nc.vector.reciprocal(r[:], st["den_g"])
        for bb in range(st["w0"], st["w1"]):
            slot = bb - st["w0"]
            lg = st["lg"].pop(bb)
            if scales_on_scalar:
                nc.scalar.mul(lg[:], lg[:], r[:, slot : slot + 1])
            else:
                nc.vector.tensor_scalar_mul(
                    lg[:], lg[:], r[:, slot : slot + 1]
                )
            b0 = bb * BB
            nc.scalar.dma_start(out[b0 : b0 + BB, :], lg[:])

    # event table: emit right after stage1_slab(bb, s).  Placement rules:
    # - stage2a(bb) only at slots the wave's AllReduce has surely finished
    #   by (its matmuls head-of-line block the PE queue while waiting).
    # - den collapses (DVE) only at slots the exps have surely finished by.
    # - den of waves 0/1 rides the avg AR two waves later; dens of waves
    #   2/3 go in the small post-stream AR.
    events = {
        (0, 0): [warmup_cc],
        (0, 2): [lambda: nc.sync.dma_start(eb[:], emb)],
        (1, 0): [lambda: nc.sync.dma_start(wo[:], w_out)],
        (3, 0): [lambda: run_ar(0)],
        (5, 1): [lambda: run_ar(1)],
        (6, 0): [lambda: stage2a(0, collapse=False)],
        (6, 1): [lambda: (
            stage2a(1, collapse=False),
            den_collapse(0),
        )],
        (6, 2): [lambda: (
            stage2a(2, collapse=False),
            den_collapse(1), den_collapse(2),
        )],
        (6, 3): [lambda: stage2a(3, collapse=False)],
        (6, 4): [lambda: stage2a(4, collapse=False)],
        (7, 0): [lambda: run_ar(2, ride_den=0)],
    }

    # the slot clock (one "ms" per slab slot) pins the Tile scheduler's
    # modeled issue order: without it the scheduler hoists stage-2 matmuls
    # ahead of stage-1 tile bursts into PE idle gaps, where they head-of-
    # line block the PE queue on a not-yet-finished collective readback.
    for bb in range(N_BB):
        for s in range(5):
            with tc.tile_wait_until(bb * 5 + s):
                stage1_slab(bb, s)
                for fn in events.get((bb, s), []):
                    fn()

    # tail: avg{7}+den{3,4} AR, blocks 5-7 epilogue, final den AR, outs
    tail = [
        lambda: den_collapse(3),
        lambda: den_collapse(4),
        lambda: run_ar(3, ride_den=1, bounce_eng=nc.sync),
        # stage2b(0) only here: its DVE recip waits on AR2; emitted any
        # earlier it would gate the den{3,4} collapses and thus AR3
        lambda: stage2b(0),
        lambda: stage2a(5, collapse=False),
        lambda: stage2a(6, collapse=False),
        lambda: stage2a(7, collapse=False),
        lambda: stage2b(1),
        lambda: den_collapse(5),
        lambda: den_collapse(6),
        lambda: den_collapse(7),
        lambda: run_final_den_ar(),
        lambda: stage2b(2, scales_on_scalar=False),
        lambda: stage2b(3, scales_on_scalar=False),
    ]
    for k, fn in enumerate(tail):
        with tc.tile_wait_until(40 + 0.3 * k):
            fn()


def build(num_devices=N_CORES):
    nc = bacc.Bacc(
        "TRN2",
        target_bir_lowering=False,
        debug=False,
        num_devices=num_devices,
        num_swdge_queues=4,
    )
    batch = nc.dram_tensor(
        "batch", [B_FULL, C, VS], F32, kind="ExternalInput"
    ).ap()
    # emb arrives host-pre-shuffled: row p holds emb rows {p, 128+p, ...}
    # so the SBUF load is one contiguous 16 KB descriptor per partition
    emb = nc.dram_tensor(
        "emb", [128, N_TILES * D], F32, kind="ExternalInput"
    ).ap()
    w_out = nc.dram_tensor("w_out", [D, VS], F32, kind="ExternalInput").ap()
    out = nc.dram_tensor("out", [B_FULL, VS], BF16, kind="ExternalOutput").ap()
    with tile.TileContext(nc) as tc:
        _cbow_kernel(tc, out, batch, emb, w_out)
    nc.compile()
    return nc


_NC = None


def _build_cached():
    global _NC
    if _NC is None:
        _NC = build()
    return _NC


def _run(batch, emb, w_out, trace=False, **kwargs):
    from concourse.bass_utils import run_bass_kernel_spmd

    nc = _build_cached()
    batch = np.ascontiguousarray(np.asarray(batch, dtype=np.float32))
    emb = np.asarray(emb, dtype=np.float32)
    w_out = np.asarray(w_out, dtype=np.float32)
    in_maps = []
    for i in range(N_CORES):
        v0 = i * VS
        emb_pad = np.zeros((VS_PAD, D), dtype=np.float32)
        emb_pad[:VS] = emb[v0 : v0 + VS]
        # shuffle to [128, n_tiles*128]: partition p holds rows {p, 128+p, ...}
        emb_shuf = np.ascontiguousarray(
            emb_pad.reshape(N_TILES, 128, D).transpose(1, 0, 2).reshape(
                128, N_TILES * D
            )
        )
        in_maps.append(
            {
                "batch": np.ascontiguousarray(batch[:, :, v0 : v0 + VS]),
                "emb": emb_shuf,
                "w_out": np.ascontiguousarray(w_out[:, v0 : v0 + VS]),
            }
        )
    res = run_bass_kernel_spmd(
        nc, in_maps, core_ids=list(range(N_CORES)), trace=trace, **kwargs
    )
    out = np.concatenate(
        [r["out"].astype(np.float32) for r in res.results], axis=1
    )
    return out, res


def kernel(batch, emb, w_out):
    out, _ = _run(batch, emb, w_out, trace=False)
    return out
